# revision 28
# baseline (speedup 1.0000x reference)
"""Self-contained Trainium2 Bass kernel for nn_MultiHeadAttention_71528385347884.

Strategy: head tensor-parallel across 8 cores (2 heads/core), with QKV
projection, causal attention, and the output projection FUSED into one
streaming pipeline over 256-wide token subtiles:

  - QKV projection (PE-heavy) is emitted in 16-chunk units; after every
    chunk one attention unit of the PREVIOUS subtile is emitted, so the
    exp/softmax work (ACT/DVE/Pool) hides under QKV matmuls.
  - Attention units are software-pipelined: unit k emits scores(k) and
    AV(k-1), so the in-order PE queue never waits on an exp in flight.
  - Causal structure: diagonal 512x512 super-blocks are emitted first
    (d0 full width with PSUM start, then d1 trimmed), score matmuls and
    exp are trimmed to the causally-valid t-tail, and the 128-wide
    triangle is zeroed in place by affine_select on the Pool engine.
  - The softmax denominator never costs PE per block: probs accumulate
    across s-blocks on DVE (bf16), one small ones-matmul per t-tile
    contracts the partitions.
  - RoPE via host-side A/B weight-column packing (no cross-partition ops).
  - Output projection exploits the reference's scrambled
    transpose(0,2,1,3).reshape(B,T,C): each core produces disjoint output
    rows -> host gather is pure concatenation. Projection units fill PE
    bubbles at subtile boundaries once a batch's attention is complete.
"""

import math
import numpy as np
import ml_dtypes

# ---- problem constants (hardcoded; kernel.py must not read spec/reference) ----
B = 2
T = 2048          # sequence length per batch
C = 2048          # model dim
Dh = 128          # head dim
N_HEAD = 16
N_CORES = 8
H_LOCAL = 2       # heads per core
ROPE_BASE = 10000.0
SCALE = 1.0 / math.sqrt(Dh)

BF16 = ml_dtypes.bfloat16

SW = 256          # legacy constant kept for Cfg asserts
TW = 512          # streaming q-tile width (t columns); PSUM bank = 512 f32
EW = 256          # output-projection e-slice width


class Cfg:
    def __init__(self, B=B, T=T, C=C):
        assert T % SW == 0 and C % 128 == 0
        self.B = B
        self.T = T
        self.C = C
        self.NCC = C // 128        # contraction chunks for qkv matmuls
        self.BT = B * T
        self.NST = T // SW         # 256-wide subtiles per batch
        self.GRP = C // Dh         # tokens folded per output row by the reshape
        self.TAU = T // self.GRP   # output rows per (b, h); must be 128
        assert self.TAU == 128
        self.ET = max(1, C // 512)  # 512-wide e-tiles of the output
        self.JQK = 4 * 128         # qA,qB,kA,kB feature blocks
        self.JV = H_LOCAL * 128


FULL = Cfg()


# =====================================================================
# Device program builder
# =====================================================================

def build_nc(cfg: Cfg, debug=False, repeat=1):
    import concourse.bass as bass
    import concourse.mybir as mybir
    import concourse.tile as tile
    from concourse import bacc
    from collections import deque

    f32 = mybir.dt.float32
    bf16 = mybir.dt.bfloat16
    Exp = mybir.ActivationFunctionType.Exp
    Copy = mybir.ActivationFunctionType.Copy

    nc = bacc.Bacc(None, target_bir_lowering=False, debug=debug)

    xt_d = nc.dram_tensor("xt", [128, cfg.NCC, cfg.BT], bf16, kind="ExternalInput")
    wqk_d = nc.dram_tensor("wqk", [128, cfg.NCC, cfg.JQK], bf16, kind="ExternalInput")
    wv_d = nc.dram_tensor("wv", [128, cfg.NCC, cfg.JV], bf16, kind="ExternalInput")
    wp_d = nc.dram_tensor("wp", [128, cfg.GRP, cfg.C], bf16, kind="ExternalInput")
    cc2_d = nc.dram_tensor("cc2", [128, cfg.T], bf16, kind="ExternalInput")
    spm_d = nc.dram_tensor("spm", [128, cfg.T], bf16, kind="ExternalInput")
    smp_d = nc.dram_tensor("smp", [128, cfg.T], bf16, kind="ExternalInput")
    swp_d = nc.dram_tensor("swp", [128, 128], bf16, kind="ExternalInput")
    out_d = nc.dram_tensor("out", [cfg.B, H_LOCAL, 128, cfg.C], f32,
                           kind="ExternalOutput")

    NT = cfg.T // TW          # 512-wide q tiles per batch
    ET = cfg.C // EW          # 256-wide output e-slices

    with tile.TileContext(nc) as tc:
        with tc.tile_pool(name="persist", bufs=1) as persist:
            wqk_sb = persist.tile([128, cfg.NCC, cfg.JQK], bf16, name="wqk_sb",
                                  tag="wqk_sb")
            wv_sb = persist.tile([128, cfg.NCC, cfg.JV], bf16, name="wv_sb",
                                 tag="wv_sb")
            cc2_sb = persist.tile([128, cfg.T], bf16, name="cc2_sb", tag="cc2_sb")
            spm_sb = persist.tile([128, cfg.T], bf16, name="spm_sb", tag="spm_sb")
            smp_sb = persist.tile([128, cfg.T], bf16, name="smp_sb", tag="smp_sb")
            ones_sb = persist.tile([128, 128], bf16, name="ones_sb", tag="ones_sb")
            swp_sb = persist.tile([128, 128], bf16, name="swp_sb", tag="swp_sb")

            wstep = max(1, cfg.NCC // 4)

            def preload_w(q):
                nc.sync.dma_start(wv_sb[:, q:q + wstep, :],
                                  wv_d[:, q:q + wstep, :])
                nc.sync.dma_start(wqk_sb[:, q:q + wstep, :],
                                  wqk_d[:, q:q + wstep, :])

            preload_w(0)
            nc.sync.dma_start(swp_sb[:], swp_d[:])
            nc.vector.memset(ones_sb[:], 1.0)

            qh_sb, kh_sb, v_sb, attn_sb = {}, {}, {}, {}
            for b in range(cfg.B):
                for hl in range(H_LOCAL):
                    qh_sb[(b, hl)] = persist.tile([128, cfg.T], bf16,
                                                  name=f"qh_{b}_{hl}",
                                                  tag=f"qh_{b}_{hl}")
                    kh_sb[(b, hl)] = persist.tile([128, cfg.T], bf16,
                                                  name=f"kh_{b}_{hl}",
                                                  tag=f"kh_{b}_{hl}")
                    v_sb[(b, hl)] = persist.tile(
                        [128, cfg.T // 128, 128], bf16,
                        name=f"v_{b}_{hl}", tag=f"v_{b}_{hl}")
                    attn_sb[(b, hl)] = persist.tile(
                        [128, cfg.T], bf16,
                        name=f"at_{b}_{hl}", tag=f"at_{b}_{hl}")

            with (
                tc.tile_pool(name="xs_pool", bufs=2) as xs_pool,
                tc.tile_pool(name="rtmp", bufs=4) as rtmp,
                tc.tile_pool(name="stgp", bufs=2) as stg_pool,
                tc.tile_pool(name="vfm", bufs=3) as vfm_pool,
                tc.tile_pool(name="probs", bufs=6) as probs_pool,
                tc.tile_pool(name="dacc", bufs=4) as dacc_pool,
                tc.tile_pool(name="recp", bufs=2) as rec_pool,
                tc.tile_pool(name="wpe", bufs=3) as wpe_pool,
                tc.tile_pool(name="ostg", bufs=2) as ostg_pool,
                # PSUM: every accumulation group owns a full 2KB bank;
                # po/pp/ps never share tags (slot-release deadlocks)
                tc.tile_pool(name="pjp", bufs=1, space="PSUM") as pjp,
                tc.tile_pool(name="sps", bufs=3, space="PSUM") as sps,
                tc.tile_pool(name="ops", bufs=2, space="PSUM") as ops,
                tc.tile_pool(name="pps", bufs=1, space="PSUM") as pps,
            ):
                fill_c = deque()
                fill_d = deque()
                pending_d = {}   # key -> unit list, armed when C(b) drained
                c_left = {}      # key -> #units left for (b, NT-1)
                RESERVE_C = 4
                RESERVE_D = 4

                def pop_c():
                    if fill_c:
                        tag, th = fill_c.popleft()
                        th()
                        if tag in c_left:
                            c_left[tag] -= 1
                            if c_left[tag] == 0 and tag in pending_d:
                                fill_d.extend(pending_d.pop(tag))
                        return True
                    return False

                def chunk_fill():
                    if len(fill_c) > RESERVE_C:
                        pop_c()
                    elif len(fill_d) > RESERVE_D:
                        fill_d.popleft()()

                for rep in range(repeat):
                    xs_tiles = {}

                    def emit_xdma(b, tt):
                        xs = xs_pool.tile([128, cfg.NCC, TW], bf16,
                                          name=f"xs_{rep}_{b}_{tt}", tag="xs")
                        bt0 = b * cfg.T + tt * TW
                        half = cfg.NCC // 2
                        nc.sync.dma_start(xs[:, 0:half, :],
                                          xt_d[:, 0:half, bt0:bt0 + TW])
                        nc.sync.dma_start(xs[:, half:cfg.NCC, :],
                                          xt_d[:, half:cfg.NCC, bt0:bt0 + TW])
                        xs_tiles[(b, tt)] = xs

                    def c_units(b, tt):
                        """Software-pipelined attention units for q-tile tt."""
                        gl = slice(tt * TW, (tt + 1) * TW)
                        n_sc = 4 * tt + 4
                        order = [4 * tt + d for d in range(4)] + \
                            list(range(4 * tt))
                        # po/acc are allocated lazily at POP time (inside the
                        # first unit): allocating at queue time lets a
                        # projection unit popped in between grab a buffer
                        # whose release depends on not-yet-emitted work -> a
                        # true scheduling deadlock.
                        po, acc = [], []
                        prs = {}

                        def alloc_tiles():
                            po.extend(ops.tile([128, TW], f32,
                                               name=f"po_{rep}_{b}_{tt}_{h}",
                                               tag="po")
                                      for h in range(2))
                            acc.extend(dacc_pool.tile([128, TW], bf16,
                                                      name=f"acc_{rep}_{b}_{tt}_{h}",
                                                      tag="acc")
                                       for h in range(2))

                        def emit_scores(i):
                            sc = order[i]
                            d = sc - 4 * tt
                            c0 = d * 128 if d >= 0 else 0
                            last = (i == n_sc - 1)
                            sl = slice(sc * 128, (sc + 1) * 128)
                            pr = []
                            for h in range(2):
                                ph = sps.tile([128, TW], f32,
                                              name=f"ps_{rep}_{b}_{tt}_{sc}_{h}",
                                              tag="ps")
                                p = probs_pool.tile([128, TW], bf16,
                                                    name="pr", tag="pr")
                                pr.append(p)
                                nc.tensor.matmul(
                                    ph[:, c0:TW],
                                    kh_sb[(b, h)][:, sl],
                                    qh_sb[(b, h)][:, tt * TW + c0:
                                                  (tt + 1) * TW],
                                    start=True, stop=True)
                                nc.scalar.activation(p[:, c0:TW],
                                                     ph[:, c0:TW], Exp,
                                                     scale=SCALE)
                                if d >= 0:
                                    if last:
                                        # close-out block: zero stale left
                                        # half AND triangle in one select so
                                        # the full-width AV adds zeros there
                                        nc.gpsimd.affine_select(
                                            p[:], p[:],
                                            pattern=[[1, TW]],
                                            compare_op=mybir.AluOpType.is_ge,
                                            fill=0.0,
                                            base=-c0,
                                            channel_multiplier=-1)
                                    else:
                                        nc.gpsimd.affine_select(
                                            p[:, c0:c0 + 128],
                                            p[:, c0:c0 + 128],
                                            pattern=[[1, 128]],
                                            compare_op=mybir.AluOpType.is_ge,
                                            fill=0.0,
                                            base=0,
                                            channel_multiplier=-1)
                            prs[i] = pr

                        def emit_av(i):
                            sc = order[i]
                            d = sc - 4 * tt
                            last = (i == n_sc - 1)
                            c0 = d * 128 if (d >= 0 and not last) else 0
                            pr = prs.pop(i)
                            for h in range(2):
                                nc.tensor.matmul(
                                    po[h][:, c0:TW], v_sb[(b, h)][:, sc, :],
                                    pr[h][:, c0:TW],
                                    start=(i == 0), stop=last)
                                if i == 0:
                                    nc.vector.tensor_copy(acc[h][:], pr[h][:])
                                elif c0:
                                    nc.vector.tensor_add(acc[h][:, c0:TW],
                                                         acc[h][:, c0:TW],
                                                         pr[h][:, c0:TW])
                                else:
                                    nc.vector.tensor_add(acc[h][:], acc[h][:],
                                                         pr[h][:])

                        def emit_tail():
                            for h in range(2):
                                pd = sps.tile([128, TW], f32,
                                              name=f"pd_{rep}_{b}_{tt}_{h}",
                                              tag="ps")
                                nc.tensor.matmul(pd[:], ones_sb[:], acc[h][:],
                                                 start=True, stop=True)
                                rec = rec_pool.tile([128, TW], f32,
                                                    name=f"rec_{rep}_{b}_{tt}_{h}",
                                                    tag="rec")
                                nc.vector.reciprocal(rec[:], pd[:])
                                nc.vector.tensor_mul(attn_sb[(b, h)][:, gl],
                                                     po[h][:], rec[:])

                        for i in range(n_sc):
                            def unit(i=i):
                                if i == 0:
                                    alloc_tiles()
                                emit_scores(i)
                                if i > 0:
                                    emit_av(i - 1)
                            yield unit

                        def final():
                            emit_av(n_sc - 1)
                            emit_tail()
                        yield final

                    def d_units(b):
                        cur = {}

                        def load_wpe(et):
                            t = wpe_pool.tile([128, cfg.GRP, EW], bf16,
                                              name=f"wpe_{rep}_{b}_{et}",
                                              tag="wpe")
                            nc.sync.dma_start(t[:],
                                              wp_d[:, :, et * EW:(et + 1) * EW])
                            cur[et] = t

                        def prep():
                            for et in range(3):
                                load_wpe(et)
                        yield prep
                        for et in range(ET):
                            for hl in range(H_LOCAL):
                                def unit(et=et, hl=hl, b=b):
                                    if hl == 0 and et + 3 < ET:
                                        load_wpe(et + 3)
                                    el = slice(et * EW, (et + 1) * EW)
                                    pp = pps.tile([128, TW], f32,
                                                  name=f"pp_{rep}_{b}_{et}_{hl}",
                                                  tag="pp")
                                    at = attn_sb[(b, hl)]
                                    w = cur[et]
                                    for u in range(cfg.GRP):
                                        nc.tensor.matmul(
                                            pp[:, 0:EW], at[:, u::cfg.GRP],
                                            w[:, u, :],
                                            start=(u == 0),
                                            stop=(u == cfg.GRP - 1))
                                    og = ostg_pool.tile(
                                        [128, EW], f32,
                                        name=f"og_{rep}_{b}_{et}_{hl}",
                                        tag="og")
                                    # DVE, not ACT: an ACT-queued og-copy can
                                    # deadlock (po slot <- og <- ACT behind an
                                    # exp whose scores follow the stuck matmul)
                                    nc.vector.tensor_copy(og[:], pp[:, 0:EW])
                                    nc.sync.dma_start(out_d[b, hl, :, el],
                                                      og[:])
                                yield unit

                    # ---------- streaming emission ----------
                    emit_xdma(0, 0)
                    for b in range(cfg.B):
                        for tt in range(NT):
                            if tt + 1 < NT:
                                emit_xdma(b, tt + 1)
                            elif b + 1 < cfg.B:
                                emit_xdma(b + 1, 0)
                            xs = xs_tiles.pop((b, tt))
                            gl = slice(tt * TW, (tt + 1) * TW)

                            def rope_pair(stg, d0t, d1t):
                                # bf16 rope from the staged copy: 2x DVE rate
                                m1 = rtmp.tile([128, TW], bf16, name="m1",
                                               tag="rt")
                                m2 = rtmp.tile([128, TW], bf16, name="m2",
                                               tag="rt")
                                m3 = rtmp.tile([128, TW], bf16, name="m3",
                                               tag="rt")
                                m4 = rtmp.tile([128, TW], bf16, name="m4",
                                               tag="rt")
                                nc.vector.tensor_mul(m1[:], stg[:, 0, :],
                                                     cc2_sb[:, gl])
                                nc.vector.tensor_mul(m2[:], stg[:, 1, :],
                                                     spm_sb[:, gl])
                                nc.vector.tensor_mul(m3[:], stg[:, 1, :],
                                                     cc2_sb[:, gl])
                                nc.vector.tensor_mul(m4[:], stg[:, 0, :],
                                                     smp_sb[:, gl])
                                nc.vector.tensor_add(d0t[0:64, gl],
                                                     m1[0:64, :], m2[0:64, :])
                                nc.vector.tensor_add(d1t[64:128, gl],
                                                     m1[64:128, :], m2[64:128, :])
                                rb = rtmp.tile([128, TW], bf16, name="rb",
                                               tag="rtb")
                                nc.vector.tensor_add(rb[:], m3[:], m4[:])
                                # swap partition halves on PE: out[m]=rb[m+64]
                                ps_sw = sps.tile([128, TW], f32,
                                                 name="ps_sw", tag="ps")
                                nc.tensor.matmul(ps_sw[:], swp_sb[:], rb[:],
                                                 start=True, stop=True)
                                nc.scalar.activation(d0t[64:128, gl],
                                                     ps_sw[64:128, :], Copy)
                                nc.scalar.activation(d1t[0:64, gl],
                                                     ps_sw[0:64, :], Copy)

                            # ---- three 2-bank passes: q, k, v ----
                            for pidx, pas in enumerate(("q", "k", "v")):
                                pj = pjp.tile([128, 2, TW], f32,
                                              name=f"pj_{rep}_{b}_{tt}_{pas}",
                                              tag="pj")
                                for ccs in range(cfg.NCC):
                                    for j in range(2):
                                        if pas == "v":
                                            w_ap = wv_sb[:, ccs,
                                                         j * 128:(j + 1) * 128]
                                        else:
                                            j4 = 2 * pidx + j
                                            w_ap = wqk_sb[:, ccs,
                                                          j4 * 128:
                                                          (j4 + 1) * 128]
                                        nc.tensor.matmul(
                                            pj[:, j, :], w_ap, xs[:, ccs, :],
                                            start=(ccs == 0),
                                            stop=(ccs == cfg.NCC - 1))
                                    if pidx == 0 and rep == 0 and b == 0 \
                                            and tt == 0 and ccs == 0:
                                        for q in range(wstep, cfg.NCC, wstep):
                                            preload_w(q)
                                        nc.sync.dma_start(cc2_sb[:, 0:TW],
                                                          cc2_d[:, 0:TW])
                                        nc.sync.dma_start(spm_sb[:, 0:TW],
                                                          spm_d[:, 0:TW])
                                        nc.sync.dma_start(smp_sb[:, 0:TW],
                                                          smp_d[:, 0:TW])
                                    if pidx == 0 and rep == 0 and b == 0 \
                                            and tt == 1 and ccs == 0:
                                        nc.sync.dma_start(cc2_sb[:, TW:cfg.T],
                                                          cc2_d[:, TW:cfg.T])
                                        nc.sync.dma_start(spm_sb[:, TW:cfg.T],
                                                          spm_d[:, TW:cfg.T])
                                        nc.sync.dma_start(smp_sb[:, TW:cfg.T],
                                                          smp_d[:, TW:cfg.T])
                                    chunk_fill()
                                if pas == "v":
                                    for hl in range(H_LOCAL):
                                        vf = vfm_pool.tile(
                                            [128, TW], bf16,
                                            name=f"vf_{rep}_{b}_{tt}_{hl}",
                                            tag="vf")
                                        nc.scalar.activation(vf[:],
                                                             pj[:, hl, :],
                                                             Copy)
                                        nc.sync.dma_start_transpose(
                                            v_sb[(b, hl)][:,
                                                          4 * tt:4 * tt + 4,
                                                          :],
                                            vf[:])
                                else:
                                    # bf16 stage copy frees both banks; rope
                                    # runs from SBUF off the critical path
                                    stg = stg_pool.tile(
                                        [128, 2, TW], bf16,
                                        name=f"stg_{rep}_{b}_{tt}_{pas}",
                                        tag="stg")
                                    nc.vector.tensor_copy(stg[:], pj[:])
                                    hs = qh_sb if pas == "q" else kh_sb
                                    rope_pair(stg, hs[(b, 0)], hs[(b, 1)])
                                    pop_c()
                                    pop_c()
                            # queue this tile's attention, then boundary fills
                            key = (rep, b) if tt == NT - 1 else None
                            units = [(key, u) for u in c_units(b, tt)]
                            if key is not None:
                                c_left[key] = len(units)
                                pending_d[key] = list(d_units(b))
                            fill_c.extend(units)
                            for _ in range(4):
                                pop_c()
                            if len(fill_d) > RESERVE_D:
                                fill_d.popleft()()
                # drain after the last rep
                n = 0
                while fill_c:
                    pop_c()
                    n += 1
                    if n % 3 == 0 and fill_d:
                        fill_d.popleft()()
                while fill_d:
                    fill_d.popleft()()

    nc.compile()
    return nc


# =====================================================================
# Host-side input prep / output gather
# =====================================================================

def _part_major(a2d, ncc):
    """[ncc*128, F] -> [128, ncc, F] with row r = chunk*128 + p."""
    F = a2d.shape[1]
    return np.ascontiguousarray(
        a2d.reshape(ncc, 128, F).transpose(1, 0, 2))


def make_trig(cfg: Cfg):
    pos = np.arange(cfg.T, dtype=np.float64)[None, :]        # [1,T]
    j = np.arange(64, dtype=np.float64)[:, None]             # [64,1]
    inv = ROPE_BASE ** (-2.0 * j / Dh)
    ang = pos * inv                                          # [64,T]
    sin = np.sin(ang).astype(np.float32)
    cos = np.cos(ang).astype(np.float32)
    cc2 = np.concatenate([cos, cos], axis=0).astype(BF16)    # [128,T]
    spm = np.concatenate([-sin, sin], axis=0).astype(BF16)
    smp = np.concatenate([sin, -sin], axis=0).astype(BF16)
    return cc2, spm, smp


def make_in_maps(x, w_qkv, w_proj, cfg: Cfg = FULL, n_cores=N_CORES,
                 n_head=N_HEAD):
    x = np.asarray(x, np.float32)
    w_qkv = np.asarray(w_qkv, np.float32)
    w_proj = np.asarray(w_proj, np.float32)
    Cm = cfg.C

    xT = np.ascontiguousarray(x.reshape(cfg.BT, Cm).T)       # [C, BT]
    xt = _part_major(xT, cfg.NCC).astype(BF16)
    wp = _part_major(w_proj, cfg.GRP).astype(BF16)
    cc2, spm, smp = make_trig(cfg)
    swp = np.roll(np.eye(128, dtype=np.float32), 64, axis=0).astype(BF16)

    wq = w_qkv[:, 0:Cm]
    wk = w_qkv[:, Cm:2 * Cm]
    wv_all = w_qkv[:, 2 * Cm:3 * Cm]

    in_maps = []
    for c in range(n_cores):
        h0, h1 = 2 * c, 2 * c + 1
        q0 = wq[:, h0 * 128:(h0 + 1) * 128]
        q1 = wq[:, h1 * 128:(h1 + 1) * 128]
        k0 = wk[:, h0 * 128:(h0 + 1) * 128]
        k1 = wk[:, h1 * 128:(h1 + 1) * 128]
        qA = np.concatenate([q0[:, 0:64], q1[:, 64:128]], axis=1)
        qB = np.concatenate([q0[:, 64:128], q1[:, 0:64]], axis=1)
        kA = np.concatenate([k0[:, 0:64], k1[:, 64:128]], axis=1)
        kB = np.concatenate([k0[:, 64:128], k1[:, 0:64]], axis=1)
        wqk = _part_major(
            np.concatenate([qA, qB, kA, kB], axis=1), cfg.NCC).astype(BF16)
        wv = _part_major(
            np.concatenate([wv_all[:, h0 * 128:(h0 + 1) * 128],
                            wv_all[:, h1 * 128:(h1 + 1) * 128]], axis=1),
            cfg.NCC).astype(BF16)
        in_maps.append(dict(xt=xt, wqk=wqk, wv=wv, wp=wp,
                            cc2=cc2, spm=spm, smp=smp, swp=swp))
    return in_maps


def gather(outs, cfg: Cfg = FULL):
    """outs: per-core [B, H_LOCAL, 128, C] -> full [B, T, C]."""
    rows = np.concatenate(
        [o.reshape(cfg.B, H_LOCAL * 128, cfg.C) for o in outs], axis=1)
    return np.ascontiguousarray(rows.reshape(cfg.B, cfg.T, cfg.C))


# =====================================================================
# Public entry point
# =====================================================================

_NC_CACHE = {}


def get_nc(debug=False):
    key = ("full", debug)
    if key not in _NC_CACHE:
        _NC_CACHE[key] = build_nc(FULL, debug=debug)
    return _NC_CACHE[key]


def kernel(x, w_qkv, w_proj):
    from concourse.bass_utils import run_bass_kernel_spmd
    nc = get_nc()
    in_maps = make_in_maps(x, w_qkv, w_proj)
    res = run_bass_kernel_spmd(nc, in_maps, list(range(N_CORES)))
    return gather([res.results[c]["out"] for c in range(N_CORES)])
